# revision 21
# baseline (speedup 1.0000x reference)
"""Trainium2 Bass kernel for the differentiable-Kalman-filter loss.

Math: the reference runs a T=100000-step linear recurrence
  x_{i+1} = M x_i + K obs[i-1],  eps_i = obs[i] - C x_{i+1},  M = A - K C
and accumulates yvar = sum outer(eps_i) + decaying P-terms, loss = slogdet(yvar/T).
rho(M) ~ 0.963, so the recurrence has ~400-step memory: eps becomes a truncated
causal convolution of obs.  Each core computes eps for a 12160-row slab via a
two-level blocked conv (B=16 within-block taps as one 512x512 triangular matmul,
block-boundary states from J=16 block-level taps), then accumulates the Gram
E^T E on-chip.  The first W=2720 rows + the tiny P-series are computed exactly
on host in f64 (they need the exact initial transient and cost ~nothing).

Perf layout (v3): input halves of each tensor ride both HWDGE rings in
need-order (rings drain round-robin, so per-ring order alone cannot
prioritize); PE kept busy from ~7us via junk matmuls on a memset tile (HAM
clock gate re-throttles after ~3.4us idle); late eps groups open with the xbt
term so the last tile's ADD->gram tail is short.
"""
import numpy as np

T, N, B, J, W, NCORES = 100000, 32, 16, 8, 2720, 8
R = (T - W) // NCORES       # rows per core = 12160
NB = R // B                 # 760 blocks per core
PSI = NB + J                # 776 panel columns (incl halo)
NTS = 6                     # s-tiles per core
PS = [128, 128, 128, 128, 128, 120]
J0S = [0, 4]

# The obs term rides inside trilh as a +32-shifted identity, so no obs
# block tiles are uploaded; step 15 comes from one tiny matmul per tile.
# trilh chunk kc is all-zero below col 128*kc-32, so chunks are packed to
# their nonzero widths [512, 416, 288, 160] (both DMA bytes and conv
# columns shrink).
TRI_C0 = [0, 96, 224, 352]          # first nonzero out-col per kc chunk
TRI_W = [512, 416, 288, 160]
# ringA: gmat | pans kc0,kc1 | trilh kc0
# ringB: halo | pans kc2,kc3 | trilh kc1 | trilh kc3 | dstk
# SWDGE:  smalls, trilh kc2 (keeps both HWDGE rings light)
A_PANS, A_TRI0, A_COLS = 128, 1664, 2176
B_PANS, B_TRI1, B_TRI3, B_DSTK, B_COLS = 128, 1664, 2080, 2240, 2368
X_COLS = 288
S_COLS = 544                # smalls: identb(32) | cmn(512)

_PROG_CACHE = {}


def _build_device_consts(A64, C64, K64):
    import ml_dtypes
    bf16 = ml_dtypes.bfloat16
    M = A64 - K64 @ C64
    Mp = [np.eye(N)]
    for _ in range(B + 1):
        Mp.append(M @ Mp[-1])
    H = [C64 @ Mp[k] @ K64 for k in range(B)]
    TrilHneg = np.zeros((512, 512))
    for r in range(B):
        for t in range(r, B):
            TrilHneg[r*N:(r+1)*N, t*N:(t+1)*N] = -H[t - r].T
    # obs injection: obs[block, c] = panel[block, c+32] for steps 0..14
    for c in range(480):
        TrilHneg[c + 32, c] += 1.0
    Gmat = np.zeros((512, N))
    for r in range(B):
        Gmat[r*N:(r+1)*N, :] = (Mp[B-1-r] @ K64).T
    CMn = np.zeros((N, 512))
    for t in range(B):
        CMn[:, t*N:(t+1)*N] = -(C64 @ Mp[t+1]).T
    MB = Mp[B]
    D = [np.eye(N)]
    for _ in range(J - 1):
        D.append(MB @ D[-1])
    tri4 = TrilHneg.reshape(4, 128, 512).transpose(1, 0, 2)   # [128, 4, 512]
    trilh = np.ascontiguousarray(np.concatenate(
        [tri4[:, kc, TRI_C0[kc]:512] for kc in range(4)], axis=1)).astype(bf16)
    gmat = np.ascontiguousarray(Gmat.reshape(4, 128, N).transpose(1, 0, 2).reshape(128, 128)).astype(bf16)
    cmn = np.ascontiguousarray(CMn).astype(bf16)
    dstk = np.zeros((128, 128))
    for jg in range(J // 4):
        for rho in range(4):
            dstk[32*rho:32*rho+32, 32*jg:32*jg+32] = D[4*jg + rho].T
    dstk = dstk.astype(bf16)
    identb = np.eye(32).astype(bf16)
    return trilh, gmat, cmn, dstk, identb


def _host_exact(obs, A64, C64, K64, x0, Psqrt0):
    """f64 exact: P-series + outer(obs0) + eps outers for i < W."""
    obs64 = obs.astype(np.float64)
    M = A64 - K64 @ C64
    Y = np.outer(obs64[0], obs64[0])
    P = Psqrt0.astype(np.float64)
    for _ in range(4000):
        CP = C64 @ P
        Y += CP @ CP.T
        P = M @ P
        if np.abs(P).max() < 1e-16:
            break
    x = x0.astype(np.float64)
    for i in range(W):
        o_prev = obs64[i - 1] if i > 0 else obs64[T - 1]
        x = M @ x + K64 @ o_prev
        eps = obs64[i] - C64 @ x
        Y += np.outer(eps, eps)
    return Y


def _patch_tile_drain():
    """This walrus build allows only one sem wait per Drain; split the
    TileContext tail drain's waits across multiple drain instructions."""
    import concourse.tile as tile
    from concourse.vector_clock import ScopedClock
    if getattr(tile.TileContext, "_kf_drain_patched", False):
        return
    def _drain_and_barrier(self, tick_clock, wait_clock):
        nc = self.nc
        drain_inst = nc.sync.drain()
        wait_clock.add_sem_waits(drain_inst.ins, ScopedClock({None: tick_clock.global_clock}))
        si = drain_inst.ins.sync_info
        waits = list(si.on_wait or [])
        if len(waits) > 1:
            si.on_wait = waits[:1]
            for i in range(1, len(waits)):
                extra = nc.sync.drain()
                esi = extra.ins.sync_info
                if esi is None:
                    extra.ins.sync_info = type(si)(on_wait=waits[i:i+1], on_update=[])
                else:
                    esi.on_wait = waits[i:i+1]
        nc.all_engine_barrier(sem_only=True)
        assert self.sems is not None
        popped = nc._tile_sem_poison_stack.pop()
        assert popped is self._sem_poison
        nc.clear_and_free_semaphores(list(self.sems.allocated().values()))
    tile.TileContext._drain_and_barrier = _drain_and_barrier
    tile.TileContext._kf_drain_patched = True


def _split_multi_waits(nc):
    """This walrus build encodes at most one sem wait per instruction; hoist
    extra waits onto NoOps inserted just before in the same engine stream."""
    import concourse.mybir as mybir
    for func in nc.m.functions:
        for blk in func.blocks:
            insts = blk.instructions
            out, changed = [], False
            for inst in insts:
                si = inst.sync_info
                waits = list(si.on_wait) if si and si.on_wait else []
                if len(waits) > 1:
                    changed = True
                    for k, w in enumerate(waits[:-1]):
                        out.append(mybir.InstNoOp(
                            name=f"{inst.name}-hw{k}", engine=inst.engine,
                            bass_nofuse=True,
                            sync_info=mybir.SyncInfo(on_wait=[w], on_update=[])))
                    si.on_wait = [waits[-1]]
                out.append(inst)
            if changed:
                blk.instructions = out


def build_program(debug=False):
    import concourse.bass as bass
    import concourse.mybir as mybir
    import concourse.tile as tile
    _patch_tile_drain()
    f32 = mybir.dt.float32
    bf16 = mybir.dt.bfloat16

    nc = bass.Bass()
    A_in = nc.declare_dram_parameter("bulka", [128, A_COLS], bf16, isOutput=False)
    B_in = nc.declare_dram_parameter("bulkb", [128, B_COLS], bf16, isOutput=False)
    S_in = nc.declare_dram_parameter("smalls", [32, S_COLS], bf16, isOutput=False)
    X_in = nc.declare_dram_parameter("aux", [128, X_COLS], bf16, isOutput=False)
    yout = nc.declare_dram_parameter("yout", [128, 512], f32, isOutput=True)

    with tile.TileContext(nc) as tc:
        with (
            tc.tile_pool(name="big", bufs=1) as bpool,
            tc.tile_pool(name="work", bufs=1) as wpool,
            tc.tile_pool(name="etile", bufs=6) as epool,
            tc.tile_pool(name="ps2k", bufs=2, space="PSUM") as ppool,
            tc.tile_pool(name="epsum", bufs=5, space="PSUM") as eppool,
            tc.tile_pool(name="gramps", bufs=1, space="PSUM") as gpool,
        ):
            RA = bpool.tile([128, A_COLS], bf16)
            RB = bpool.tile([128, B_COLS], bf16)
            RX = bpool.tile([128, X_COLS], bf16)
            S = wpool.tile([32, S_COLS], bf16)
            warm = wpool.tile([128, 128], bf16)
            zero512 = wpool.tile([128, 512], bf16)

            # ---- warm tiles via memset (no DMA dep), smalls on SWDGE ring
            nc.gpsimd.memset(warm[:], 0.0)
            nc.gpsimd.memset(zero512[:], 0.0)
            nc.gpsimd.dma_start(S[:], S_in[:])
            nc.gpsimd.dma_start(RX[:], X_in[:])
            # ---- bulk inputs: mirrored need-order on both HWDGE rings,
            # chunked so each piece's completion sem (~1.8us receipt) fires
            # as soon as its bytes are in rather than at ring end
            for c0, c1 in [(0, A_PANS), (A_PANS, A_PANS + 512),
                           (A_PANS + 512, A_PANS + 1024),
                           (A_PANS + 1024, A_TRI0), (A_TRI0, A_COLS)]:
                nc.sync.dma_start(RA[:, c0:c1], A_in[:, c0:c1])
            for c0, c1 in [(0, B_PANS), (B_PANS, B_PANS + 512),
                           (B_PANS + 512, B_PANS + 1024),
                           (B_PANS + 1024, B_TRI1), (B_TRI1, B_DSTK),
                           (B_DSTK, B_COLS)]:
                nc.scalar.dma_start(RB[:, c0:c1], B_in[:, c0:c1])

            gmat = RA[:, 0:128]
            identb = S[:, 0:32]
            cmn = S[:, 32:544]

            def pans(kc, c0, c1):
                if kc < 2:
                    return RA[:, A_PANS + 768*kc + c0 : A_PANS + 768*kc + c1]
                return RB[:, B_PANS + 768*(kc-2) + c0 : B_PANS + 768*(kc-2) + c1]

            tri_base = {0: (RA, A_TRI0), 1: (RB, B_TRI1),
                        2: (RX, 0), 3: (RB, B_TRI3)}

            def trilh(kc, r0, r1):
                ring, base = tri_base[kc]
                return ring[:, base + r0 : base + r1]

            gram_ps = gpool.tile([128, 512], f32)
            gramA = gram_ps[:, 0:256]
            gramB = gram_ps[:, 256:512]

            def junk(n):
                # PE keep-warm: HAM un-throttles only under sustained activity
                for _ in range(n):
                    nc.tensor.matmul(gram_ps[:, 0:128], lhsT=warm[:], rhs=warm[:],
                                     start=True, stop=True, skip_group_check=True)

            eps_list = [None] * NTS
            esb_list = [None] * NTS
            for st in range(NTS - 1):
                eps_list[st] = eppool.tile([128, 512], f32, tag="epsum",
                                           name=f"eps{st}")

            junk(16)

            # ---- gT [32, 776]: halo states + main panel states (split 512|264)
            gtA = ppool.tile([32, 512], f32, tag="ps2k")
            gtB = ppool.tile([32, 248 + J], f32, tag="ps2k")
            for kc in range(4):
                nc.tensor.matmul(gtA[:, 0:J],
                                 lhsT=gmat[:, 32*kc : 32*kc+32],
                                 rhs=RB[:, 32*kc : 32*kc + J],
                                 start=(kc == 0), stop=False)
            # pre-zero the eps banks during the DMA wait so the conv
            # accumulations skip the start=True full-row-zero premium
            for st in range(NTS - 1):
                nc.tensor.matmul(eps_list[st][:, :], lhsT=warm[:],
                                 rhs=zero512[:], start=True, stop=True,
                                 skip_group_check=True)
            junk(8)
            for kc in range(4):
                nc.tensor.matmul(gtA[:, J : 512],
                                 lhsT=gmat[:, 32*kc : 32*kc+32],
                                 rhs=pans(kc, 0, 512 - J),
                                 start=False, stop=(kc == 3))
                nc.tensor.matmul(gtB[:, 0:248 + J],
                                 lhsT=gmat[:, 32*kc : 32*kc+32],
                                 rhs=pans(kc, 512 - J, 760),
                                 start=(kc == 0), stop=(kc == 3))

            def conv_st(st, start, c0=0, c1=512):
                p = PS[st]
                eps_ps = eps_list[st]
                for kc in range(4):
                    a, b = max(c0, TRI_C0[kc]), c1
                    if a >= b:
                        if not start and kc == 3:
                            raise AssertionError("stop mm must not be empty")
                        continue
                    nc.tensor.matmul(eps_ps[:p, a:b],
                                     lhsT=pans(kc, 128*st, 128*st + p),
                                     rhs=trilh(kc, a - TRI_C0[kc], b - TRI_C0[kc]),
                                     start=(start and kc == 0 and st == 5),
                                     stop=(not start and kc == 3),
                                     skip_group_check=(st != 5))

            def xcmn_st(st, start, xbt):
                p = PS[st]
                nc.tensor.matmul(eps_list[st][:p, :],
                                 lhsT=xbt[:, 128*st : 128*st+p],
                                 rhs=cmn[:, :],
                                 start=start, stop=(not start),
                                 skip_group_check=(st != 5))

            def obstail_st(st, stop):
                # obs step-15 term: panel position 0:32 of the NEXT block col
                p = PS[st]
                nc.tensor.matmul(eps_list[st][:p, 480:512],
                                 lhsT=RA[0:32, A_PANS + 128*st + 1 :
                                         A_PANS + 128*st + 1 + p],
                                 rhs=identb[:],
                                 start=False, stop=stop,
                                 skip_group_check=(st != 5))

            def stage_st(st, c0=0, c1=512):
                p = PS[st]
                if esb_list[st] is None:
                    esb_list[st] = epool.tile([128, 512], bf16, tag="etile",
                                              name=f"esb{st}")
                esb = esb_list[st]
                if c0 < 288:
                    nc.vector.tensor_copy(esb[:p, c0:288],
                                          eps_list[st][:p, c0:288])
                if c1 > 288:
                    nc.scalar.copy(esb[:p, 288:c1], eps_list[st][:p, 288:c1])

            def gram_st(st, first, gs=(0, 1, 2, 3)):
                p = PS[st]
                esb = esb_list[st]
                for g in gs:
                    # start=True zeroes the full 2KB bank row (both halves of
                    # the shared gram bank), so only the very first matmul of
                    # the whole gram accumulation may set it.
                    nc.tensor.matmul(gram_ps[:, 128*g : 128*g+128],
                                     lhsT=esb[:p, 128*g : 128*g+128],
                                     rhs=esb[:p, 128*g : 128*g+128],
                                     start=(first and g == 0),
                                     stop=(st == NTS - 1 and g in (1, 3)),
                                     skip_group_check=True)

            conv_st(0, start=True)
            obstail_st(0, stop=False)

            # ---- gts bf16 [32, 776]
            gts = wpool.tile([32, PSI], bf16)
            nc.vector.tensor_copy(gts[:, 0:512], gtA[:])
            nc.scalar.copy(gts[:, 512:PSI], gtB[:])

            # ---- gS [128, 776]: group rho = gT shifted right by rho cols
            gsA = ppool.tile([128, 512], f32, tag="ps2k")
            gsB = ppool.tile([128, 248 + J], f32, tag="ps2k")
            for rho in range(4):
                tp = (0, 32 * rho) if rho else None
                nc.tensor.matmul(gsA[32*rho : 32*rho+32, rho:512],
                                 lhsT=identb[:],
                                 rhs=gts[:, 0 : 512-rho],
                                 start=True, stop=True, tile_position=tp)
                nc.tensor.matmul(gsB[32*rho : 32*rho+32, 0:248 + J],
                                 lhsT=identb[:],
                                 rhs=gts[:, 512-rho : PSI-rho],
                                 start=True, stop=True, tile_position=tp)

            conv_st(1, start=True)
            obstail_st(1, stop=False)

            gss = wpool.tile([128, PSI], bf16)
            nc.vector.tensor_copy(gss[:, 0:512], gsA[:])
            nc.scalar.copy(gss[:, 512:PSI], gsB[:])

            # ---- XbT [32, 760]: sum_j D_j g_{s+15-j} via 4 tap-groups of 4
            xbtA = ppool.tile([32, 512], f32, tag="ps2k")
            xbtB = ppool.tile([32, 248], f32, tag="ps2k")
            for jg, j0 in enumerate(J0S):
                nc.tensor.matmul(xbtA[:, 0:512],
                                 lhsT=RB[:, B_DSTK + 32*jg : B_DSTK + 32*jg+32],
                                 rhs=gss[:, (J-1-j0) : (J-1-j0) + 512],
                                 start=(j0 == 0), stop=(j0 == J0S[-1]))
                nc.tensor.matmul(xbtB[:, 0:248],
                                 lhsT=RB[:, B_DSTK + 32*jg : B_DSTK + 32*jg+32],
                                 rhs=gss[:, (J-1-j0) + 512 : (J-1-j0) + 760],
                                 start=(j0 == 0), stop=(j0 == J0S[-1]))

            conv_st(2, start=True)
            obstail_st(2, stop=False)
            conv_st(3, start=True)
            obstail_st(3, stop=False)
            eps_list[5] = ppool.tile([128, 512], f32, tag="ps2k", name="eps5")

            xbt = wpool.tile([32, NB], bf16)
            nc.vector.tensor_copy(xbt[:, 0:512], xbtA[:])
            nc.scalar.copy(xbt[:, 512:NB], xbtB[:])

            # ---- sts 0-4 close with the xbt term so their convs are only
            # data-gated; st5 opens with it so its ADD->gram tail is short
            conv_st(4, start=True)
            obstail_st(4, stop=False)
            xcmn_st(0, start=False, xbt=xbt)
            stage_st(0)
            xcmn_st(1, start=False, xbt=xbt)
            stage_st(1)
            conv_st(5, start=True)
            obstail_st(5, stop=False)
            xcmn_st(2, start=False, xbt=xbt)
            stage_st(2)
            gram_st(0, first=True)
            xcmn_st(3, start=False, xbt=xbt)
            stage_st(3)
            gram_st(1, first=False)
            xcmn_st(4, start=False, xbt=xbt)
            stage_st(4)
            gram_st(2, first=False)
            xcmn_st(5, start=False, xbt=xbt)
            # last tile: stage/gram split by halves so each gram half closes
            # early and its copy+DMA overlaps the other half's tail
            ysb = wpool.tile([128, 512], f32)
            stage_st(5, 0, 288)
            stage_st(5, 288, 512)
            gram_st(3, first=False)
            gram_st(4, first=False)
            gram_st(5, first=False, gs=(0, 1))
            nc.scalar.copy(ysb[:, 0:256], gramA)
            nc.sync.dma_start(yout[:, 0:256], ysb[:, 0:256])
            gram_st(5, first=False, gs=(2, 3))
            nc.vector.tensor_copy(ysb[:, 256:512], gramB)
            nc.scalar.dma_start(yout[:, 256:512], ysb[:, 256:512])

    _split_multi_waits(nc)
    return nc


def _core_inputs(obs, c, consts):
    """Host-side layout prep for one core: pack ringA / ringB / smalls."""
    import ml_dtypes
    bf16 = ml_dtypes.bfloat16
    trilh, gmat, cmn, dstk, identb = consts
    start = W + c * R
    hb = J * B + 1                                      # halo rows + 1
    flat = obs[start - hb : start + R]
    # panel rows (shifted by -1 obs row): s in [0, 760)
    pm = np.zeros((768, 512), np.float32)
    pm[:NB] = flat[hb - 1 : hb - 1 + R].reshape(NB, 512)
    # block col NB, position 0:32 feeds the step-15 obs term of block NB-1
    pm[NB, 0:32] = flat[hb - 1 + R]
    ptm = pm.reshape(768, 4, 128).transpose(2, 1, 0)    # [128, 4, 768]
    pth = np.zeros((128, 4, 32), np.float32)
    ph = flat[0 : J * B].reshape(J, 512)                # halo panel rows
    pth[:, :, :J] = ph.reshape(J, 4, 128).transpose(2, 1, 0)

    trilh32 = trilh.astype(np.float32)     # packed [128, 1376]
    toff = np.cumsum([0] + TRI_W)           # chunk offsets in packed trilh
    bulka = np.zeros((128, A_COLS), np.float32)
    bulka[:, 0:128] = gmat.astype(np.float32)
    bulka[:, A_PANS:A_TRI0] = ptm[:, 0:2, :].reshape(128, 1536)
    bulka[:, A_TRI0:A_COLS] = trilh32[:, toff[0]:toff[1]]
    aux = trilh32[:, toff[2]:toff[3]]

    bulkb = np.zeros((128, B_COLS), np.float32)
    bulkb[:, 0:128] = pth.reshape(128, 128)
    bulkb[:, B_PANS:B_TRI1] = ptm[:, 2:4, :].reshape(128, 1536)
    bulkb[:, B_TRI1:B_TRI3] = trilh32[:, toff[1]:toff[2]]
    bulkb[:, B_TRI3:B_DSTK] = trilh32[:, toff[3]:toff[4]]
    bulkb[:, B_DSTK:B_COLS] = dstk.astype(np.float32)

    smalls = np.zeros((32, S_COLS), np.float32)
    smalls[:, 0:32] = identb.astype(np.float32)
    smalls[:, 32:544] = cmn.astype(np.float32)

    return {"bulka": bulka.astype(bf16), "bulkb": bulkb.astype(bf16),
            "smalls": smalls.astype(bf16), "aux": np.ascontiguousarray(aux).astype(bf16)}


def kernel(observations, A, C, K, x0, Psqrt0, _trace=False, _trace_kwargs=None):
    obs = np.ascontiguousarray(observations, np.float32)
    A64 = np.asarray(A, np.float64)
    C64 = np.asarray(C, np.float64)
    K64 = np.asarray(K, np.float64)

    consts = _build_device_consts(A64, C64, K64)
    Y = _host_exact(obs, A64, C64, K64, np.asarray(x0), np.asarray(Psqrt0))

    if "prog" not in _PROG_CACHE:
        _PROG_CACHE["prog"] = build_program()
    nc = _PROG_CACHE["prog"]

    in_maps = [_core_inputs(obs, c, consts) for c in range(NCORES)]

    from concourse.bass_utils import run_bass_kernel_spmd
    kw = dict(_trace_kwargs or {})
    res = run_bass_kernel_spmd(nc, in_maps, list(range(NCORES)), trace=_trace, **kw)

    for c in range(NCORES):
        G = np.asarray(res.results[c]["yout"], np.float64)
        for g in range(4):
            for tau in range(4):
                Y += G[32*tau:32*tau+32, 128*g+32*tau : 128*g+32*tau+32]
    loss = np.linalg.slogdet(Y / T)[1]
    out = np.float32(loss)
    if _trace:
        return out, res
    return out
